# revision 1
# baseline (speedup 1.0000x reference)
"""Trainium2 Bass kernel for nn_CrossAggregator (gnn_message_passing).

out[g,o] = self[g]·W1[o,:] + ea_g^T A_o eb_g,  g=(b,m), A_o = W[o,128:].reshape(128,128)
ea/eb = masked means over 32 neighbors (t=0 / t=1).

Design (per core, batch/8 data-parallel, G=512 rows):
- ea-side: masked-mean + partition-broadcast FUSED into K=32 f32r matmuls
  (stationary = all-ones/32, row-group selected via tile_position) -> PSUM earep.
- eb-side: masked-mean via K=128 f32r matmuls with a banded selector (BIG) as
  stationary -> ebT [j,g] directly in PSUM.
- mask multiplies on GPSIMD (tensor_tensor, stride-0 mask broadcast), f32r out.
- outer-product chunks Pt on DVE: pt[j, (i,g)] = ebT[j,g] * earep_i[j,g], f32r out.
- main contraction on PE: psum_out[o,g] += W2chunk_i^T @ pt_i  (f32r, N=512).
- host does only layout transforms (shard/permute/pack) + output transpose.
"""
import sys
import numpy as np

for _p in ("/opt/trn_rl_repo", "/root/.axon_site/_ro/trn_rl_repo"):
    if _p not in sys.path:
        sys.path.insert(0, _p)

B, M, TWO, NN, D = 1024, 4, 2, 32, 128
OUT = 128
NCORES = 8
BC = B // NCORES          # batches per core
G = BC * M                # 512 rows per core
NIG = D // 4              # 32 slabs of 4 i's

_CACHE = {}


def _build_nc():
    import os
    import concourse.bacc as bacc_mod
    import concourse.mybir as mybir
    from concourse.tile import TileContext

    F32 = mybir.dt.float32
    F32R = mybir.dt.float32r
    MUL = mybir.AluOpType.mult

    nc = bacc_mod.Bacc(None)
    SKIP_GPS = bool(int(os.environ.get("SKIP_GPS", "0")))
    SKIP_PT = bool(int(os.environ.get("SKIP_PT", "0")))
    SKIP_MAIN = bool(int(os.environ.get("SKIP_MAIN", "0")))

    d_naR = nc.declare_dram_parameter("naR", [NIG, 128, G], F32, isOutput=False)
    d_nbR = nc.declare_dram_parameter("nbR", [NIG, 128, G], F32, isOutput=False)
    d_maskA = nc.declare_dram_parameter("maskA", [128, G], F32, isOutput=False)
    d_maskB = nc.declare_dram_parameter("maskB", [128, G], F32, isOutput=False)
    d_selfT = nc.declare_dram_parameter("selfT", [D, G], F32, isOutput=False)
    d_W1 = nc.declare_dram_parameter("W1a", [D, OUT], F32, isOutput=False)
    d_W2 = nc.declare_dram_parameter("W2p", [NIG, D, 4 * OUT], F32, isOutput=False)
    d_BIG = nc.declare_dram_parameter("BIG", [128, 252], F32, isOutput=False)
    d_ones = nc.declare_dram_parameter("ones32", [128, 128], F32, isOutput=False)
    d_out = nc.declare_dram_parameter("outT", [OUT, G], F32, isOutput=True)

    with TileContext(nc) as tc:
        with (
            tc.tile_pool(name="const", bufs=1) as cpool,
            tc.tile_pool(name="nb_raw", bufs=2) as nbpool,
            tc.tile_pool(name="nb_msk", bufs=2) as nbmpool,
            tc.tile_pool(name="na_raw", bufs=2) as napool,
            tc.tile_pool(name="na_msk", bufs=2) as nampool,
            tc.tile_pool(name="w2", bufs=4) as w2pool,
            tc.tile_pool(name="pt", bufs=2) as ptpool,
            tc.tile_pool(name="misc", bufs=1) as mpool,
            tc.tile_pool(name="ps_ebt", bufs=1, space="PSUM") as ps_ebt,
            tc.tile_pool(name="ps_rep", bufs=2, space="PSUM") as ps_rep,
            tc.tile_pool(name="ps_out", bufs=1, space="PSUM") as ps_out,
        ):
            # constants
            big_t = cpool.tile([128, 252], F32R, tag="big")
            nc.sync.dma_start(out=big_t[:], in_=d_BIG[:].bitcast(F32R))
            ones_t = cpool.tile([128, 128], F32R, tag="ones")
            nc.sync.dma_start(out=ones_t[:], in_=d_ones[:].bitcast(F32R))
            maskA_t = cpool.tile([128, G], F32, tag="ma")
            nc.sync.dma_start(out=maskA_t[:], in_=d_maskA[:])
            maskB_t = cpool.tile([128, G], F32, tag="mb")
            nc.sync.dma_start(out=maskB_t[:], in_=d_maskB[:])
            selfT_t = cpool.tile([D, G], F32R, tag="sT")
            nc.sync.dma_start(out=selfT_t[:], in_=d_selfT[:].bitcast(F32R))
            w1_t = cpool.tile([D, OUT], F32R, tag="w1")
            nc.sync.dma_start(out=w1_t[:], in_=d_W1[:].bitcast(F32R))

            _loop_n = int(os.environ.get("KERNEL_LOOP", "0"))
            _amp = int(os.environ.get("KERNEL_AMP", "1"))
            from contextlib import nullcontext
            _ctx = tc.For_i(0, _loop_n, 1) if _loop_n else nullcontext()
            with _ctx:
              for _rep in range(_amp):
                # ---- EB phase: ebT[j, g] in PSUM ----
                p_ebt = ps_ebt.tile([128, G], F32, tag="ebt")
                for sg in range(8):  # 4 slabs per DMA
                    nb4 = nbpool.tile([128, 4 * G], F32R if SKIP_GPS else F32, tag="nb4")
                    nc.sync.dma_start(
                        out=nb4[:].rearrange("p (s c) -> p s c", s=4),
                        in_=d_nbR[4 * sg : 4 * sg + 4].bitcast(F32R if SKIP_GPS else F32).rearrange("s p c -> p s c"),
                    )
                    if SKIP_GPS:
                        mb4 = nb4
                    else:
                        mb4 = nbmpool.tile([128, 4 * G], F32R, tag="mb4")
                        nc.gpsimd.tensor_tensor(
                            out=mb4[:].rearrange("p (s c) -> p s c", s=4),
                            in0=nb4[:].rearrange("p (s c) -> p s c", s=4),
                            in1=maskB_t[:][:, None, :].broadcast_to([128, 4, G]),
                            op=MUL,
                        )
                    for u in range(4):
                        jg = 4 * sg + u
                        nc.tensor.matmul(
                            p_ebt[:],
                            big_t[:, 124 - 4 * jg : 252 - 4 * jg],
                            mb4[:, G * u : G * (u + 1)],
                            start=(jg == 0),
                            stop=(jg == NIG - 1),
                        )
                ebT_sb = mpool.tile([128, G], F32, tag="ebsb")
                nc.scalar.copy(out=ebT_sb[:], in_=p_ebt[:])

                # ---- MAIN phase ----
                p_out = ps_out.tile([OUT, G], F32, tag="out")
                nc.tensor.matmul(p_out[:], w1_t[:], selfT_t[:], start=True, stop=False)

                ma2_tiles = {}
                for k in range(64):  # pair k covers i = 2k, 2k+1
                    ig = k // 2
                    if ig % 2 == 0 and k % 2 == 0:
                        na2 = napool.tile([128, 2 * G], F32R if SKIP_GPS else F32, tag="na2")
                        nc.sync.dma_start(
                            out=na2[:].rearrange("p (s c) -> p s c", s=2),
                            in_=d_naR[ig : ig + 2].bitcast(F32R if SKIP_GPS else F32).rearrange("s p c -> p s c"),
                        )
                        if SKIP_GPS:
                            ma2 = na2
                        else:
                            ma2 = nampool.tile([128, 2 * G], F32R, tag="ma2")
                            nc.gpsimd.tensor_tensor(
                                out=ma2[:].rearrange("p (s c) -> p s c", s=2),
                                in0=na2[:].rearrange("p (s c) -> p s c", s=2),
                                in1=maskA_t[:][:, None, :].broadcast_to([128, 2, G]),
                                op=MUL,
                            )
                        ma2_tiles[ig] = ma2
                        ma2_tiles[ig + 1] = ma2
                    if k % 2 == 0:
                        w2_t = w2pool.tile([D, 4 * OUT], F32R, tag="w2t")
                        nc.sync.dma_start(
                            out=w2_t[:], in_=d_W2[k // 2].bitcast(F32R)
                        )
                    ma2 = ma2_tiles[ig]
                    slab_off = (ig % 2) * G  # which slab within the pair tile
                    rep = ps_rep.tile([128, 2 * G], F32, tag="rep")
                    for u in range(2):
                        isub = 2 * (k % 2) + u
                        nc.tensor.matmul(
                            rep[:, G * u : G * (u + 1)],
                            ones_t[32 * isub : 32 * isub + 32, :],
                            ma2[32 * isub : 32 * isub + 32, slab_off : slab_off + G],
                            start=True,
                            stop=True,
                            tile_position=(32 * isub, 0),
                        )
                    if SKIP_PT:
                        pt2 = ma2
                    else:
                        pt2 = ptpool.tile([128, 2 * G], F32R, tag="pt2")
                        if k % int(os.environ.get("PTMOD", "4")) == int(os.environ.get("PTMOD", "4")) - 1 and not SKIP_GPS:
                            rep_sb = ptpool.tile([128, 2 * G], F32, tag="repsb")
                            nc.scalar.copy(out=rep_sb[:], in_=rep[:])
                            nc.gpsimd.tensor_tensor(
                                out=pt2[:].rearrange("p (a c) -> p a c", a=2),
                                in0=ebT_sb[:][:, None, :].broadcast_to([128, 2, G]),
                                in1=rep_sb[:].rearrange("p (a c) -> p a c", a=2),
                                op=MUL,
                            )
                        else:
                            nc.vector.tensor_tensor(
                                out=pt2[:].rearrange("p (a c) -> p a c", a=2),
                                in0=ebT_sb[:][:, None, :].broadcast_to([128, 2, G]),
                                in1=rep[:].rearrange("p (a c) -> p a c", a=2),
                                op=MUL,
                            )
                    for u in (range(0) if SKIP_MAIN else range(2)):
                        i = 2 * k + u
                        w2col = (i % 4) * OUT
                        nc.tensor.matmul(
                            p_out[:],
                            w2_t[:, w2col : w2col + OUT],
                            pt2[:, G * u : G * (u + 1)],
                            start=False,
                            stop=(k == 63 and u == 1),
                        )

                out_sb = mpool.tile([OUT, G], F32, tag="osb")
                nc.scalar.copy(out=out_sb[:], in_=p_out[:])
                nc.sync.dma_start(out=d_out[:], in_=out_sb[:])

    nc.finalize()
    return nc


def _host_prep(self_vectors, neighbor_vectors, masks, W):
    f32 = np.float32
    sv = np.ascontiguousarray(self_vectors, dtype=f32)
    nv = np.ascontiguousarray(neighbor_vectors, dtype=f32)
    mk = np.ascontiguousarray(masks, dtype=f32)
    Wf = np.ascontiguousarray(W, dtype=f32)

    # per-core packs
    nvc = nv.reshape(NCORES, G, TWO, NN, D)          # [c, g, t, n, d]
    naR = np.ascontiguousarray(
        nvc[:, :, 0].transpose(0, 3, 2, 1).reshape(NCORES, NIG, 128, G)
    )  # [c, ig, (isub,n), g]
    nbR = np.ascontiguousarray(
        nvc[:, :, 1].transpose(0, 3, 2, 1).reshape(NCORES, NIG, 128, G)
    )
    mkc = mk.reshape(NCORES, G, TWO, NN)             # [c, g, t, n]
    mA = mkc[:, :, 0].transpose(0, 2, 1)             # [c, n, g]
    mB = mkc[:, :, 1].transpose(0, 2, 1)
    maskA = np.ascontiguousarray(
        np.broadcast_to(mA[:, None], (NCORES, 4, NN, G)).reshape(NCORES, 128, G)
    )
    maskB = np.ascontiguousarray(
        np.broadcast_to(mB[:, None], (NCORES, 4, NN, G)).reshape(NCORES, 128, G)
    )
    selfT = np.ascontiguousarray(
        sv.reshape(NCORES, G, D).transpose(0, 2, 1)
    )  # [c, d, g]

    # shared weights
    W1a = np.ascontiguousarray(Wf[:, :D].T)                       # [d, o]
    w2 = Wf[:, D:].reshape(OUT, D, D)                             # [o, i, j]
    W2p = np.ascontiguousarray(
        w2.transpose(1, 2, 0)                                     # [i, j, o]
        .reshape(NIG, 4, D, OUT)                                  # [ig, isub, j, o]
        .transpose(0, 2, 1, 3)                                    # [ig, j, isub, o]
        .reshape(NIG, D, 4 * OUT)
    )
    BIG = np.zeros((128, 252), f32)
    r = np.arange(128)
    BIG[r, 124 + r // 32] = 1.0 / 32.0
    ones32 = np.full((128, 128), 1.0 / 32.0, f32)

    in_maps = []
    for c in range(NCORES):
        in_maps.append(
            {
                "naR": naR[c],
                "nbR": nbR[c],
                "maskA": maskA[c],
                "maskB": maskB[c],
                "selfT": selfT[c],
                "W1a": W1a,
                "W2p": W2p,
                "BIG": BIG,
                "ones32": ones32,
            }
        )
    return in_maps


def kernel(self_vectors, neighbor_vectors, masks, W, b):
    from concourse.bass_utils import run_bass_kernel_spmd

    if "nc" not in _CACHE:
        _CACHE["nc"] = _build_nc()
    nc = _CACHE["nc"]
    in_maps = _host_prep(self_vectors, neighbor_vectors, masks, W)
    results = run_bass_kernel_spmd(nc, in_maps, list(range(NCORES))).results
    out = np.empty((B, M, OUT), np.float32)
    for c in range(NCORES):
        out[c * BC : (c + 1) * BC] = (
            results[c]["outT"].T.reshape(BC, M, OUT)
        )
    out += np.asarray(b, np.float32)[None, None, :]
    return out

